# revision 14
# baseline (speedup 1.0000x reference)
"""Trainium2 Bass kernel for nn_Net_71554155151864 (e3nn-style GNN message-passing layer).

Strategy (v2):
 - Shard edges across 8 cores BY GRAPH (2 graphs/core): e3LayerNorm per-graph
   segment statistics are core-local; no cross-core collective at all.
 - Feature-major on-device layout; vector (1o) channels m-major (m0|m1|m2).
 - The FullyConnectedTensorProduct (edge_fea x one_hot) kron operand is
   precomputed on the HOST in fp8-e4m3 and contracted on the PE with
   DoubleRow matmuls (2 one-hot slots per pass).
 - All matmuls bf16/fp8 (fp32 matmuls run at 1/4 rate on TRN2).
 - Node-feature gathers via gpsimd.dma_gather(transpose=True) from a bf16
   node table with 768-byte rows.
 - Skip connection + output in bf16; single packed DMA per tile where possible.
"""
import math
import numpy as np
import ml_dtypes

import concourse.bacc as bacc
import concourse.bass as bass
import concourse.mybir as mybir
import concourse.tile as tile
from concourse.bass_utils import run_bass_kernel_spmd
from concourse import library_config

F32 = mybir.dt.float32
BF16 = mybir.dt.float16   # f16: same speed as bf16, 8x finer mantissa
FP8 = mybir.dt.float8e4
I16 = mybir.dt.int16
NPF8 = ml_dtypes.float8_e4m3
NPBF = np.float16

N, E, G = 10000, 100000, 16
NS, NV = 128, 64
DIM = NS + 3 * NV
NSP2 = 16
FC = 128
EPS = 1e-5
NCORES = 8
ET = 512                      # edges per tile
NT = 26                       # tiles per core
EPC_P = NT * ET               # padded edges per core (13312)
NTAB_ELEM = 384               # node table row length (bf16), 768B

AL = mybir.AluOpType
AF = mybir.ActivationFunctionType
DR = mybir.MatmulPerfMode.DoubleRow

_CACHE = {}


def _mmaj(x):
    """[..., DIM] interleaved (v,m) -> m-major rows [s(128) | m0(64) | m1(64) | m2(64)]."""
    s = x[..., :NS]
    v = x[..., NS:].reshape(*x.shape[:-1], NV, 3)
    return np.concatenate([s] + [v[..., m] for m in range(3)], axis=-1)


def _bd(w):
    z = np.zeros((128, 128), w.dtype)
    z[:64, :64] = w
    z[64:, 64:] = w
    return z


def _top(w):
    z = np.zeros((128, 64), w.dtype)
    z[:64, :] = w
    return z


def _dup(w):
    """[r,64] -> [r,128] duplicated columns"""
    return np.concatenate([w, w], axis=1)


class _Pack:
    """Column-packs [rows<=128, cols] arrays into one [128, C] buffer."""
    def __init__(self, np_dtype):
        self.dt = np_dtype
        self.entries = {}
        self.col = 0

    def add(self, name, arr):
        arr = np.asarray(arr)
        assert arr.ndim == 2 and arr.shape[0] <= 128
        self.entries[name] = (self.col, arr.astype(self.dt))
        self.col += arr.shape[1]

    def buffer(self):
        buf = np.zeros((128, self.col), self.dt)
        for c, arr in self.entries.values():
            buf[: arr.shape[0], c : c + arr.shape[1]] = arr
        return buf

    def sl(self, name):
        c, arr = self.entries[name]
        return (arr.shape[0], c, arr.shape[1])


def _pack_layouts(inputs):
    """Build the packed weight buffers (shared across cores)."""
    sq2 = math.sqrt(2.0)
    pb = _Pack(NPBF)
    pf = _Pack(np.float32)
    pk8 = _Pack(NPF8)

    wpre1 = np.asarray(inputs["Wpre1"], np.float32) / math.sqrt(NV)
    pb.add("wpre0", np.asarray(inputs["Wpre0"], np.float32) / math.sqrt(NS))
    pb.add("wpre1bd", _bd(wpre1))
    pb.add("wpre1m2", wpre1)

    wss = np.asarray(inputs["Wss"], np.float32) / (math.sqrt(3 * NS) * sq2)
    wsv = np.asarray(inputs["Wsv"], np.float32) / (math.sqrt(3 * NS) * sq2)
    # os1a (128 out) per contract block i/j/v ; os1b+os2b use column-duplicated
    # 64->128 weights so the gate path comes out partition-duplicated.
    pb.add("wss1_i", wss[0:128, 0:128])
    pb.add("wss1_j", wss[128:256, 0:128])
    pb.add("wss1_v", wss[256:384, 0:128])
    pb.add("wssb_i", _dup(wss[0:128, 128:192]))
    pb.add("wssb_j", _dup(wss[128:256, 128:192]))
    pb.add("wssb_v", _dup(wss[256:384, 128:192]))
    pb.add("wsvd_i", _dup(wsv[0:128]))
    pb.add("wsvd_j", _dup(wsv[128:256]))
    pb.add("wsvd_v", _dup(wsv[256:384]))

    wvs = np.asarray(inputs["Wvs"], np.float32) / (math.sqrt(9 * NV) * sq2)
    pb.add("wvs_hi", wvs[0:128, 0:128])
    pb.add("wvs_lo", wvs[128:192, 0:128])
    pb.add("wvsb_hi", _dup(wvs[0:128, 128:192]))
    pb.add("wvsb_lo", _dup(wvs[128:192, 128:192]))

    wvv = np.asarray(inputs["Wvv"], np.float32) / (math.sqrt(3 * NV) * sq2)
    pb.add("wvv_bdi", _bd(wvv[0:64]))
    pb.add("wvv_bdj", _bd(wvv[64:128]))
    pb.add("wvv_bdv", _bd(wvv[128:192]))
    pb.add("wvv_ti", _top(wvv[0:64]))
    pb.add("wvv_tj", _top(wvv[64:128]))
    pb.add("wvv_tv", wvv[128:192])

    pb.add("wf1", np.asarray(inputs["Wf1"], np.float32))
    pb.add("wf2", np.asarray(inputs["Wf2"], np.float32))
    wf3 = np.asarray(inputs["Wf3"], np.float32)
    pb.add("wf3a", wf3[:, 0:128])
    pb.add("wf3b_d", _dup(wf3[:, 128:192]))

    pb.add("wpost0", np.asarray(inputs["Wpost0"], np.float32) / math.sqrt(NS))
    wpost1 = np.asarray(inputs["Wpost1"], np.float32) / math.sqrt(NV)
    pb.add("wpost1bd", _bd(wpost1))
    pb.add("wpost1m2", wpost1)

    i64 = np.eye(64, dtype=np.float32)
    pb.add("ll", np.vstack([i64, i64]))
    pb.add("l2", np.vstack([i64, np.zeros((64, 64), np.float32)]))
    stsel = np.zeros((128, 3, 3), np.float32)
    stsel[:, 0, 0] = 1.0; stsel[:, 1, 1] = 1.0; stsel[:, 2, 2] = 1.0
    pb.add("stsel", stsel.reshape(128, 9))
    pb.add("ident3", np.eye(3, dtype=np.float32))

    # f32 pack
    pf.add("bpre0", np.asarray(inputs["bpre0"], np.float32).reshape(128, 1))
    pf.add("bf1", np.asarray(inputs["bf1"], np.float32).reshape(64, 1))
    pf.add("bf2", np.asarray(inputs["bf2"], np.float32).reshape(64, 1))
    bf3 = np.asarray(inputs["bf3"], np.float32)
    pf.add("bf3a", bf3[0:128].reshape(128, 1))
    pf.add("bf3b_d", np.concatenate([bf3[128:192], bf3[128:192]]).reshape(128, 1))
    pf.add("bpost0", np.asarray(inputs["bpost0"], np.float32).reshape(128, 1))
    gamma_s = np.asarray(inputs["gamma_s"], np.float32)
    beta_s = np.asarray(inputs["beta_s"], np.float32)
    gamma_v = np.asarray(inputs["gamma_v"], np.float32)
    pf.add("gs_c", gamma_s.reshape(128, 1))
    pf.add("gv01_c", np.concatenate([gamma_v, gamma_v]).reshape(128, 1))
    pf.add("gv2_c", gamma_v.reshape(64, 1))
    pf.add("bs_col", beta_s.reshape(128, 1))
    pf.add("gsrep", np.tile(gamma_s[None, :], (16, 1)))
    pf.add("ones16", np.ones((16, 128), np.float32))

    pf.add("eps_c", np.full((16, 1), EPS, np.float32))

    # FCTP weights all f16 (mixed f16-stationary x fp8-moving matmul is exact)
    wsc_s = np.asarray(inputs["Wsc_s"], np.float32) / math.sqrt(NS * NSP2)
    pb.add("wsc_s", wsc_s.reshape(128, NSP2 * 128))
    wv = np.asarray(inputs["Wsc_v"], np.float32) / math.sqrt(NV * NSP2)  # [v,s,u]
    wv01 = np.stack([_bd(wv[:, s, :]) for s in range(NSP2)], axis=1)     # [128,16,128]
    pb.add("wsc_v01", wv01.reshape(128, NSP2 * 128))
    # v2: s-pairs stacked on partition halves (128-contract plain matmul)
    wv2p = np.zeros((128, 8, 64), np.float32)
    wv2p[0:64] = wv[:, 0::2, :]
    wv2p[64:128] = wv[:, 1::2, :]
    pb.add("wsc_v2p", wv2p.reshape(128, 8 * 64))
    pk8.add("pad", np.zeros((1, 16), np.float32))
    return pb, pf, pk8


def build_nc(cb_cols, cf_cols, ck_cols, sl_b, sl_f, sl_k):
    nc = bacc.Bacc("TRN2", target_bir_lowering=False, debug=False,
                   num_devices=NCORES)
    dt = nc.dram_tensor

    def inp(name, shape, d):
        return dt(name, shape, d, kind="ExternalInput").ap()

    efb = inp("efb", [128, 3, EPC_P], BF16)
    elb = inp("elb", [128, EPC_P], BF16)
    shr = inp("shr", [NT, 4 * ET], BF16)
    indb = inp("indb", [16, EPC_P], BF16)
    pk = inp("pk", [128, NT, 128], I16)
    krs = inp("krs", [128, NT, 16, ET], BF16)
    krv1 = inp("krv1", [128, NT, 16, ET], FP8)
    krv2 = inp("krv2", [128, NT, 8, ET], FP8)
    ntab = inp("ntab", [N, NTAB_ELEM], BF16)
    wpkb = inp("wpkb", [128, cb_cols], BF16)
    wpkf = inp("wpkf", [128, cf_cols], F32)
    wpk8 = inp("wpk8", [128, ck_cols], FP8)
    ivs = inp("ivs", [16, 3], F32)   # [inv_s, inv_v, unused] per graph

    out_fm = dt("out_fm", [128, 3, EPC_P], BF16, kind="ExternalOutput").ap()

    with tile.TileContext(nc) as tc:
        with (
            tc.tile_pool(name="persist", bufs=1) as pp,
            tc.tile_pool(name="loads", bufs=2) as lp,
            tc.tile_pool(name="krn", bufs=2) as kp,
            tc.tile_pool(name="gath", bufs=2) as gp,
            tc.tile_pool(name="work", bufs=1) as wp,
            tc.tile_pool(name="res", bufs=2) as rp,
            tc.tile_pool(name="shp", bufs=1) as shp,
            tc.tile_pool(name="ps", bufs=4, space="PSUM") as ps,
            tc.tile_pool(name="pz", bufs=1, space="PSUM") as pz,
            tc.tile_pool(name="pst", bufs=1, space="PSUM") as pst,
        ):
            nc.gpsimd.load_library(library_config.mlp)

            cb = pp.tile([128, cb_cols], BF16, tag="cb")
            nc.sync.dma_start(cb[:], wpkb)
            cf = pp.tile([128, cf_cols], F32, tag="cf")
            nc.sync.dma_start(cf[:], wpkf)
            ck = pp.tile([128, ck_cols], FP8, tag="ck")
            nc.sync.dma_start(ck[:], wpk8)
            c_ivs = pp.tile([16, 3], F32, tag="ivs")
            nc.sync.dma_start(c_ivs[:], ivs)

            def B(name):
                r, c, w = sl_b(name)
                return cb[0:r, c:c + w]

            def Fc(name):
                r, c, w = sl_f(name)
                return cf[0:r, c:c + w]

            def K8(name, k, outs):
                r, c, w = sl_k(name)
                # [r, 8*2*outs] -> slice pair k -> [r, 2, outs]
                return ck[0:r, c + k * 2 * outs: c + (k + 1) * 2 * outs] \
                    .rearrange("p (j u) -> p j u", j=2)

            def K8p(name):
                r, c, w = sl_k(name)
                return ck[0:r, c:c + w]

            z_s_all = pp.tile([128, EPC_P], BF16, tag="z_s_all")
            z_v01_all = pp.tile([128, EPC_P], BF16, tag="z_v01_all")
            z_v2_all = pp.tile([128, EPC_P // 2], BF16, tag="z_v2_all")
            stats_ps = pst.tile([16, 3], F32)

            # ================= PHASE 1 =================
            for t in range(NT):
                sl = slice(t * ET, (t + 1) * ET)

                ef3 = lp.tile([128, 3, ET], BF16, tag="ef3")
                nc.sync.dma_start(ef3[:], efb[:, :, sl])
                el_t = lp.tile([128, ET], BF16, tag="el_t")
                nc.sync.dma_start(el_t[:], elb[:, sl])
                sh4 = shp.tile([1, 4 * ET], BF16, tag="sh4")
                nc.sync.dma_start(sh4[:], shr[t:t + 1, :])
                pk_t = lp.tile([128, 128], I16, tag="pk_t")
                nc.sync.dma_start(pk_t[:], pk[:, t, :])
                ks_t = kp.tile([128, 16, ET], BF16, tag="ks_t")
                nc.sync.dma_start(ks_t[:], krs[:, t])
                kv1_t = kp.tile([128, 16, ET], FP8, tag="kv1_t")
                nc.sync.dma_start(kv1_t[:], krv1[:, t])
                kv2_t = kp.tile([128, 8, ET], FP8, tag="kv2_t")
                nc.sync.dma_start(kv2_t[:], krv2[:, t])

                ind4_t = pk_t[:, 0:64].bitcast(BF16).rearrange("p (a b) -> p a b", a=4)
                gix_t = pk_t[:, 64:96]
                gjx_t = pk_t[:, 96:128]
                efb_s = ef3[:, 0, :]
                efb_v01 = ef3[:, 1, :]
                efb_v2 = ef3[0:64, 2, :]

                gi = gp.tile([128, 3, ET], BF16, tag="gi")
                nc.gpsimd.dma_gather(gi[:], ntab, gix_t, ET, ET, NTAB_ELEM,
                                     transpose=True, single_packet=False)
                gj = gp.tile([128, 3, ET], BF16, tag="gj")
                nc.gpsimd.dma_gather(gj[:], ntab, gjx_t, ET, ET, NTAB_ELEM,
                                     transpose=True, single_packet=False)

                # sh broadcast tiles via gpsimd (sh4 = [sh0|sh1_0|sh1_1|sh1_2])
                shb01 = wp.tile([128, ET], BF16, tag="shb01")
                shb2 = wp.tile([128, ET], BF16, tag="shb2")
                sh0b = wp.tile([128, ET], BF16, tag="sh0b")
                nc.gpsimd.partition_broadcast(sh0b[:], sh4[0:1, 0:ET])
                nc.gpsimd.partition_broadcast(shb01[0:64, :], sh4[0:1, ET:2 * ET])
                # HW partition_broadcast cannot write at a partition offset:
                # broadcast sh1_1 at partition 0, then DMA-move to the top half
                shtmp = wp.tile([64, ET], BF16, tag="shtmp")
                nc.gpsimd.partition_broadcast(shtmp[:], sh4[0:1, 2 * ET:3 * ET])
                nc.sync.dma_start(shb01[64:128, :], shtmp[:])
                nc.gpsimd.partition_broadcast(shb2[:], sh4[0:1, 3 * ET:4 * ET])

                # lin_pre
                p = ps.tile([128, ET], F32, tag="pt")
                nc.tensor.matmul(p[:], B("wpre0"), efb_s, start=True, stop=True)
                s_sb = wp.tile([128, ET], BF16, tag="s_sb")
                nc.scalar.activation(s_sb[:], p[:], AF.Identity, bias=Fc("bpre0"))
                p = ps.tile([128, ET], F32, tag="pt")
                nc.tensor.matmul(p[:], B("wpre1bd"), efb_v01, start=True, stop=True)
                v01_sb = wp.tile([128, ET], BF16, tag="v01_sb")
                nc.scalar.copy(v01_sb[:], p[:])
                p2 = ps.tile([64, ET], F32, tag="pt")
                nc.tensor.matmul(p2[:], B("wpre1m2"), efb_v2, start=True, stop=True)
                v2_sb = wp.tile([64, ET], BF16, tag="v2_sb")
                nc.scalar.copy(v2_sb[:], p2[:])

                # radial MLP
                p2 = ps.tile([64, ET], F32, tag="pt")
                nc.tensor.matmul(p2[:], B("wf1"), el_t[:], start=True, stop=True)
                h1 = wp.tile([64, ET], BF16, tag="h1")
                nc.scalar.activation(h1[:], p2[:], AF.Silu, bias=Fc("bf1"))
                p2 = ps.tile([64, ET], F32, tag="pt")
                nc.tensor.matmul(p2[:], B("wf2"), h1[:], start=True, stop=True)
                h2 = wp.tile([64, ET], BF16, tag="h2")
                nc.scalar.activation(h2[:], p2[:], AF.Silu, bias=Fc("bf2"))
                p = ps.tile([128, ET], F32, tag="pt")
                nc.tensor.matmul(p[:], B("wf3a"), h2[:], start=True, stop=True)
                w_s = wp.tile([128, ET], BF16, tag="w_s")
                nc.scalar.activation(w_s[:], p[:], AF.Identity, bias=Fc("bf3a"))
                p = ps.tile([128, ET], F32, tag="pt")
                nc.tensor.matmul(p[:], B("wf3b_d"), h2[:], start=True, stop=True)
                w_vd = wp.tile([128, ET], BF16, tag="w_vd")
                nc.scalar.activation(w_vd[:], p[:], AF.Identity, bias=Fc("bf3b_d"))

                # d = sum_m v_in_m * sh1_m   (i/j stacked in one psum, v separate)
                prods = []
                for (src, tag) in ((gi[:, 1, :], "pd1"), (gj[:, 1, :], "pd3"),
                                   (v01_sb[:], "pd5")):
                    pr = wp.tile([128, ET], BF16, tag=tag)
                    nc.vector.tensor_tensor(pr[:], src, shb01[:], op=AL.mult)
                    prods.append(pr)
                prods2 = []
                for (src, tag) in ((gi[:, 2, :], "pd2"), (gj[:, 2, :], "pd4")):
                    pr = wp.tile([128, ET], BF16, tag=tag)
                    nc.vector.tensor_tensor(pr[:], src, shb2[:], op=AL.mult)
                    prods2.append(pr)
                pr6 = wp.tile([64, ET], BF16, tag="pd6")
                nc.vector.tensor_tensor(pr6[:], v2_sb[:], shb2[0:64, :], op=AL.mult)

                c_ll, c_l2 = B("ll"), B("l2")
                d_ij = ps.tile([128, ET], F32, tag="pt")
                nc.tensor.matmul(d_ij[0:64, :], c_ll, prods[0][:], start=True, stop=False)
                nc.tensor.matmul(d_ij[0:64, :], c_l2, prods2[0][:], start=False, stop=True)
                nc.tensor.matmul(d_ij[64:128, :], c_ll, prods[1][:], start=True, stop=False)
                nc.tensor.matmul(d_ij[64:128, :], c_l2, prods2[1][:], start=False, stop=True)
                d_v = ps.tile([64, ET], F32, tag="pt")
                nc.tensor.matmul(d_v[:], c_ll, prods[2][:], start=True, stop=False)
                nc.tensor.matmul(d_v[:], c_l2[0:64, :], pr6[:], start=False, stop=True)
                d1 = wp.tile([128, ET], BF16, tag="d1")
                nc.scalar.copy(d1[:], d_ij[:])
                d2 = wp.tile([64, ET], BF16, tag="d2")
                nc.scalar.copy(d2[:], d_v[:])

                # out_s = sh0*(s_in @ Wss) + d @ Wvs  (a: first 128; b: dup'd 64)
                os1a = ps.tile([128, ET], F32, tag="pt")
                nc.tensor.matmul(os1a[:], B("wss1_i"), gi[:, 0, :], start=True, stop=False)
                nc.tensor.matmul(os1a[:], B("wss1_j"), gj[:, 0, :], start=False, stop=False)
                nc.tensor.matmul(os1a[:], B("wss1_v"), s_sb[:], start=False, stop=True)
                os1b = ps.tile([128, ET], F32, tag="pt")
                nc.tensor.matmul(os1b[:], B("wssb_i"), gi[:, 0, :], start=True, stop=False)
                nc.tensor.matmul(os1b[:], B("wssb_j"), gj[:, 0, :], start=False, stop=False)
                nc.tensor.matmul(os1b[:], B("wssb_v"), s_sb[:], start=False, stop=True)
                os2a = ps.tile([128, ET], F32, tag="pt")
                nc.tensor.matmul(os2a[:], B("wvs_hi"), d1[:], start=True, stop=False)
                nc.tensor.matmul(os2a[:], B("wvs_lo"), d2[:], start=False, stop=True)
                os2b = ps.tile([128, ET], F32, tag="pt")
                nc.tensor.matmul(os2b[:], B("wvsb_hi"), d1[:], start=True, stop=False)
                nc.tensor.matmul(os2b[:], B("wvsb_lo"), d2[:], start=False, stop=True)

                osA = wp.tile([128, ET], F32, tag="osA")
                nc.vector.tensor_tensor(osA[:], os1a[:], sh0b[:], op=AL.mult)
                nc.vector.tensor_tensor(osA[:], osA[:], os2a[:], op=AL.add)
                osB = wp.tile([128, ET], F32, tag="osB")
                nc.vector.tensor_tensor(osB[:], os1b[:], sh0b[:], op=AL.mult)
                nc.vector.tensor_tensor(osB[:], osB[:], os2b[:], op=AL.add)

                zs_g = wp.tile([128, ET], BF16, tag="zs_g")
                nc.scalar.activation(zs_g[:], osA[:], AF.Silu)
                gate = wp.tile([128, ET], BF16, tag="gate")
                nc.scalar.activation(gate[:], osB[:], AF.Sigmoid)
                gw = wp.tile([128, ET], BF16, tag="gw")
                nc.vector.tensor_tensor(gw[:], gate[:], w_vd[:], op=AL.mult)

                # out_v = sh1_m*(s_in @ Wsv)[dup] + sh0*(v_in_m @ Wvv)
                q_ps = ps.tile([128, ET], F32, tag="pt")
                nc.tensor.matmul(q_ps[:], B("wsvd_i"), gi[:, 0, :], start=True, stop=False)
                nc.tensor.matmul(q_ps[:], B("wsvd_j"), gj[:, 0, :], start=False, stop=False)
                nc.tensor.matmul(q_ps[:], B("wsvd_v"), s_sb[:], start=False, stop=True)
                t2v01 = ps.tile([128, ET], F32, tag="pt")
                nc.tensor.matmul(t2v01[:], B("wvv_bdi"), gi[:, 1, :], start=True, stop=False)
                nc.tensor.matmul(t2v01[:], B("wvv_bdj"), gj[:, 1, :], start=False, stop=False)
                nc.tensor.matmul(t2v01[:], B("wvv_bdv"), v01_sb[:], start=False, stop=True)
                t2v2 = ps.tile([64, ET], F32, tag="pt")
                nc.tensor.matmul(t2v2[:], B("wvv_ti"), gi[:, 2, :], start=True, stop=False)
                nc.tensor.matmul(t2v2[:], B("wvv_tj"), gj[:, 2, :], start=False, stop=False)
                nc.tensor.matmul(t2v2[:], B("wvv_tv"), v2_sb[:], start=False, stop=True)

                ov01 = wp.tile([128, ET], BF16, tag="ov01")
                tmp01 = wp.tile([128, ET], F32, tag="tmp01")
                nc.vector.tensor_tensor(tmp01[:], q_ps[:], shb01[:], op=AL.mult)
                ov01f = wp.tile([128, ET], F32, tag="osA")
                nc.vector.tensor_tensor(ov01f[:], t2v01[:], sh0b[:], op=AL.mult)
                nc.vector.tensor_tensor(tmp01[:], tmp01[:], ov01f[:], op=AL.add)
                nc.vector.tensor_tensor(ov01[:], tmp01[:], gw[:], op=AL.mult)
                ov2 = wp.tile([64, ET], BF16, tag="ov2")
                tmp2 = wp.tile([64, ET], F32, tag="tmp01")
                nc.vector.tensor_tensor(tmp2[:], q_ps[0:64, :], shb2[0:64, :], op=AL.mult)
                tmp2b = wp.tile([64, ET], F32, tag="osB")
                nc.vector.tensor_tensor(tmp2b[:], t2v2[:], sh0b[0:64, :], op=AL.mult)
                nc.vector.tensor_tensor(tmp2[:], tmp2[:], tmp2b[:], op=AL.add)
                nc.vector.tensor_tensor(ov2[:], tmp2[:], gw[0:64, :], op=AL.mult)

                zs_w = wp.tile([128, ET], BF16, tag="zs_w")
                nc.vector.tensor_tensor(zs_w[:], zs_g[:], w_s[:], op=AL.mult)

                # FCTP self-connection via fp8 DoubleRow (placed late in the PE
                # stream so tile t+1's early matmuls overlap tile t's psum drain)
                z_s_ps = pz.tile([128, ET], F32, tag="z_s_ps")
                z_v01_ps = pz.tile([128, ET], F32, tag="z_v01_ps")
                z_v2_ps = pz.tile([64, ET], F32, tag="z_v2_ps")
                c_wsc_s = B("wsc_s").rearrange("p (s u) -> p s u", s=NSP2)
                c_wsc_v01 = B("wsc_v01").rearrange("p (s u) -> p s u", s=NSP2)
                for s in range(NSP2):
                    nc.tensor.matmul(z_s_ps[:], c_wsc_s[:, s, :], ks_t[:, s],
                                     start=(s == 0), stop=False)
                    nc.tensor.matmul(z_v01_ps[:], c_wsc_v01[:, s, :], kv1_t[:, s],
                                     start=(s == 0), stop=False)
                c_wv2p = B("wsc_v2p").rearrange("p (k u) -> p k u", k=8)
                for k in range(8):
                    nc.tensor.matmul(z_v2_ps[:], c_wv2p[:, k, :], kv2_t[:, k],
                                     start=(k == 0), stop=False)

                # lin_post accumulates onto the FCTP psums
                nc.tensor.matmul(z_s_ps[:], B("wpost0"), zs_w[:], start=False, stop=True)
                nc.tensor.matmul(z_v01_ps[:], B("wpost1bd"), ov01[:], start=False, stop=True)
                nc.tensor.matmul(z_v2_ps[:], B("wpost1m2"), ov2[:], start=False, stop=True)

                nc.scalar.activation(z_s_all[:, sl], z_s_ps[:], AF.Identity,
                                     bias=Fc("bpost0"))
                nc.scalar.copy(z_v01_all[:, sl], z_v01_ps[:])
                v2off = (t % 2) * 64
                sl2 = slice((t // 2) * ET, (t // 2 + 1) * ET)
                zv2_sl = z_v2_all[v2off:v2off + 64, sl2]
                nc.scalar.copy(zv2_sl, z_v2_ps[:])

                # stats: [sum(z_s); sum(z_s^2); sum(z_v^2)] per edge -> per graph
                sqs = wp.tile([128, ET], BF16, tag="sqs")
                nc.gpsimd.tensor_mul(sqs[:], z_s_all[:, sl], z_s_all[:, sl])
                sqv01 = wp.tile([128, ET], BF16, tag="sqv01")
                nc.gpsimd.tensor_mul(sqv01[:], z_v01_all[:, sl], z_v01_all[:, sl])
                sqv2 = wp.tile([64, ET], BF16, tag="sqv2")
                nc.gpsimd.tensor_mul(sqv2[:], zv2_sl, zv2_sl)
                c_stsel = B("stsel").rearrange("p (a b) -> p a b", a=3)
                st_ps = ps.tile([3, ET], F32, tag="pt")
                nc.tensor.matmul(st_ps[:], c_stsel[:, 0, :], z_s_all[:, sl], start=True, stop=False)
                nc.tensor.matmul(st_ps[:], c_stsel[:, 1, :], sqs[:], start=False, stop=False)
                nc.tensor.matmul(st_ps[:], c_stsel[:, 2, :], sqv01[:], start=False, stop=False)
                nc.tensor.matmul(st_ps[:], c_stsel[0:64, 2, :], sqv2[:], start=False, stop=True)
                st_sb = wp.tile([3, ET], BF16, tag="st_sb")
                nc.vector.tensor_copy(st_sb[:], st_ps[:])
                for c in range(4):
                    tp_ps = ps.tile([128, 3], BF16, tag="pt")
                    nc.tensor.transpose(tp_ps[:], st_sb[:, c * 128:(c + 1) * 128],
                                        B("ident3"))
                    tp_sb = wp.tile([128, 3], BF16, tag="tp_sb")
                    nc.vector.tensor_copy(tp_sb[:], tp_ps[:])
                    nc.tensor.matmul(stats_ps[:], ind4_t[:, c, :], tp_sb[:],
                                     start=(t == 0 and c == 0), stop=(t == NT - 1 and c == 3))

            # ============ stats finalize ============
            st = pp.tile([16, 3], F32, tag="st_fin")
            nc.vector.tensor_copy(st[:], stats_ps[:])
            mean = pp.tile([16, 1], F32, tag="mean")
            nc.vector.tensor_scalar(mean[:], st[:, 0:1], c_ivs[:, 0:1], None, op0=AL.mult)
            es2 = pp.tile([16, 1], F32, tag="es2")
            nc.vector.tensor_scalar(es2[:], st[:, 1:2], c_ivs[:, 0:1], None, op0=AL.mult)
            var_s = pp.tile([16, 1], F32, tag="var_s")
            nc.vector.tensor_tensor(var_s[:], mean[:], mean[:], op=AL.mult)
            nc.vector.tensor_tensor(var_s[:], es2[:], var_s[:], op=AL.subtract)
            var_v = pp.tile([16, 1], F32, tag="var_v")
            nc.vector.tensor_scalar(var_v[:], st[:, 2:3], c_ivs[:, 1:2], None, op0=AL.mult)
            rstd_s = pp.tile([16, 1], F32, tag="rstd_s")
            nc.scalar.activation(rstd_s[:], var_s[:], AF.Sqrt, bias=Fc("eps_c")[0:16, :])
            nc.vector.reciprocal(rstd_s[:], rstd_s[:])
            rstd_v = pp.tile([16, 1], F32, tag="rstd_v")
            nc.scalar.activation(rstd_v[:], var_v[:], AF.Sqrt, bias=Fc("eps_c")[0:16, :])
            nc.vector.reciprocal(rstd_v[:], rstd_v[:])

            a_l = pp.tile([16, 128], BF16, tag="a_l")
            nc.vector.tensor_scalar(a_l[:], Fc("ones16")[0:16, :], rstd_s[:, 0:1],
                                    None, op0=AL.mult)
            mrn = pp.tile([16, 1], F32, tag="mrn")
            nc.vector.tensor_scalar(mrn[:], mean[:], rstd_s[:, 0:1], -1.0,
                                    op0=AL.mult, op1=AL.mult)
            b_l = pp.tile([16, 128], BF16, tag="b_l")
            nc.vector.tensor_scalar(b_l[:, :], Fc("gsrep")[0:16, :], mrn[:, 0:1],
                                    None, op0=AL.mult)
            cc_l = pp.tile([16, 128], BF16, tag="cc_l")
            nc.vector.tensor_scalar(cc_l[:], Fc("ones16")[0:16, :], rstd_v[:, 0:1],
                                    None, op0=AL.mult)

            # ================= PHASE 2 =================
            for t in range(NT):
                sl = slice(t * ET, (t + 1) * ET)
                ind_t = lp.tile([16, ET], BF16, tag="ind_t")
                nc.sync.dma_start(ind_t[:], indb[:, sl])
                ef3b = lp.tile([128, 3, ET], BF16, tag="ef3")
                nc.sync.dma_start(ef3b[:], efb[:, :, sl])
                a_ps = ps.tile([128, ET], F32, tag="pt")
                nc.tensor.matmul(a_ps[:], a_l[:], ind_t[:], start=True, stop=True)
                b_ps = ps.tile([128, ET], F32, tag="pt")
                nc.tensor.matmul(b_ps[:], b_l[:], ind_t[:], start=True, stop=True)
                c_ps = ps.tile([128, ET], F32, tag="pt")
                nc.tensor.matmul(c_ps[:], cc_l[:], ind_t[:], start=True, stop=True)

                res3 = rp.tile([128, 3, ET], BF16, tag="res3")
                r_s = res3[:, 0, :]
                nc.vector.scalar_tensor_tensor(r_s, z_s_all[:, sl], Fc("gs_c"),
                                               a_ps[:], op0=AL.mult, op1=AL.mult)
                nc.vector.scalar_tensor_tensor(r_s, b_ps[:], Fc("bs_col"),
                                               r_s, op0=AL.add, op1=AL.add)
                nc.vector.tensor_tensor(r_s, r_s, ef3b[:, 0, :], op=AL.add)
                r_v01 = res3[:, 1, :]
                nc.vector.scalar_tensor_tensor(r_v01, z_v01_all[:, sl], Fc("gv01_c"),
                                               c_ps[:], op0=AL.mult, op1=AL.mult)
                nc.vector.tensor_tensor(r_v01, r_v01, ef3b[:, 1, :], op=AL.add)
                r_v2 = res3[0:64, 2, :]
                v2off = (t % 2) * 64
                sl2 = slice((t // 2) * ET, (t // 2 + 1) * ET)
                # operand base partitions must agree for SB inputs: gv01_c holds
                # [gamma_v; gamma_v], c_ps is partition-uniform by construction
                nc.vector.scalar_tensor_tensor(r_v2, z_v2_all[v2off:v2off + 64, sl2],
                                               Fc("gv01_c")[v2off:v2off + 64, :],
                                               c_ps[v2off:v2off + 64, :],
                                               op0=AL.mult, op1=AL.mult)
                nc.vector.tensor_tensor(r_v2, r_v2, ef3b[0:64, 2, :], op=AL.add)

                nc.sync.dma_start(out_fm[:, 0:2, sl], res3[:, 0:2, :])
                nc.sync.dma_start(out_fm[0:64, 2, sl], res3[0:64, 2, :])

    nc.compile()
    return nc


def prep_inputs(inputs):
    """Host-side: graph-shard, permute, transpose, pack per-core input dicts."""
    node_fea = np.asarray(inputs["node_fea"], np.float32)
    edge_one_hot = np.asarray(inputs["edge_one_hot"], np.float32)
    edge_sh = np.asarray(inputs["edge_sh"], np.float32)
    edge_fea = np.asarray(inputs["edge_fea"], np.float32)
    el = np.asarray(inputs["edge_length_embedded"], np.float32)
    edge_index = np.asarray(inputs["edge_index"]).astype(np.int64)
    batch = np.asarray(inputs["batch"]).astype(np.int64)

    i_idx, j_idx = edge_index[0], edge_index[1]
    batch_edge = batch[i_idx]

    # assign 2 graphs per core, balancing edge counts (largest with smallest)
    cnt_edges = np.bincount(batch_edge, minlength=G)
    order = np.argsort(-cnt_edges)
    pairs = [(order[k], order[G - 1 - k]) for k in range(G // 2)]
    core_of_graph = np.zeros(G, np.int64)
    for c, (g1, g2) in enumerate(pairs):
        core_of_graph[g1] = c
        core_of_graph[g2] = c
    core_of_edge = core_of_graph[batch_edge]

    perm = np.argsort(core_of_edge, kind="stable")
    counts = np.bincount(core_of_edge, minlength=NCORES)
    assert counts.max() <= EPC_P, f"core overflow: {counts}"
    starts = np.zeros(NCORES + 1, np.int64)
    starts[1:] = np.cumsum(counts)

    ntab = np.zeros((N, NTAB_ELEM), np.float32)
    ntab[:, :DIM] = _mmaj(node_fea)
    ntab = ntab.astype(NPBF)

    pb, pf, pk8 = _pack_layouts(inputs)
    W = {"wpkb": pb.buffer(), "wpkf": pf.buffer(), "wpk8": pk8.buffer(),
         "ntab": ntab}
    cnt = np.maximum(cnt_edges.astype(np.float32), 1.0)
    ivs = np.zeros((16, 3), np.float32)
    ivs[:, 0] = 1.0 / (cnt * NS)
    ivs[:, 1] = 1.0 / (cnt * NV * 3)
    W["ivs"] = ivs

    in_maps = []
    core_perms = []
    for c in range(NCORES):
        pidx = perm[starts[c]:starts[c + 1]]
        core_perms.append(pidx)
        ne = len(pidx)
        mm = np.zeros((EPC_P, DIM), np.float32)
        mm[:ne] = _mmaj(edge_fea[pidx])
        # efb [128, 3, EPC]: blocks s / [m0;m1] / [m2;0]
        efb = np.zeros((128, 3, EPC_P), np.float32)
        efb[:, 0, :] = mm[:, 0:128].T
        efb[:, 1, :] = mm[:, 128:256].T
        efb[0:64, 2, :] = mm[:, 256:320].T
        elc = np.zeros((EPC_P, FC), np.float32)
        elc[:ne] = el[pidx]
        shc = np.zeros((EPC_P, 4), np.float32)
        shc[:ne] = edge_sh[pidx]
        ohc = np.zeros((EPC_P, NSP2), np.float32)
        ohc[:ne] = edge_one_hot[pidx]
        be = np.zeros((EPC_P,), np.int64)
        be[:ne] = batch_edge[pidx]
        onehot = np.zeros((EPC_P, 16), np.float32)
        onehot[np.arange(ne), be[:ne]] = 1.0
        ind4c = np.ascontiguousarray(
            onehot.reshape(NT, 4, 128, 16).transpose(0, 2, 1, 3))  # [NT,128,4,16]
        iic = np.zeros((EPC_P,), np.int64)
        jjc = np.zeros((EPC_P,), np.int64)
        iic[:ne] = i_idx[pidx]
        jjc[:ne] = j_idx[pidx]

        def wrap(idx):
            # [NT,128,ET//16]: partition p holds idx[k*16 + p%16] at col k
            x = idx.reshape(NT, ET // 16, 16).transpose(0, 2, 1)  # [NT,16,32]
            return np.ascontiguousarray(np.tile(x, (1, 8, 1))).astype(np.int16)

        # pk [128, NT, 128] i16: ind4(f16 bitcast, 64) | gix 32 | gjx 32
        pkc = np.empty((128, NT, 128), np.int16)
        ind4_i16 = ind4c.astype(np.float16).reshape(NT, 128, 64).view(np.int16)
        pkc[:, :, 0:64] = ind4_i16.transpose(1, 0, 2)
        pkc[:, :, 64:96] = wrap(iic).transpose(1, 0, 2)
        pkc[:, :, 96:128] = wrap(jjc).transpose(1, 0, 2)

        def kron_raw(feat_rows, qdt):
            # feat_rows: [EPC, R] f32 -> [R, NT, 16, ET] quantized
            K3 = (feat_rows[:, None, :] * ohc[:, :, None]).astype(qdt)  # [EPC,16,R]
            R = feat_rows.shape[1]
            a = np.ascontiguousarray(K3.transpose(2, 0, 1))  # [R, EPC, 16]
            a = a.reshape(R, NT, ET, 16).transpose(0, 1, 3, 2)
            return np.ascontiguousarray(a)

        krs = kron_raw(mm[:, 0:128], NPBF)                    # [128,NT,16,ET] f16
        krv1 = kron_raw(mm[:, 128:256], NPF8)                 # [128,NT,16,ET]
        kv2 = kron_raw(mm[:, 256:320], NPF8)                  # [64,NT,16,ET]
        krv2 = np.empty((128, NT, 8, ET), NPF8)
        krv2[0:64] = kv2[:, :, 0::2, :]
        krv2[64:128] = kv2[:, :, 1::2, :]

        m = dict(
            efb=efb.astype(NPBF),
            elb=np.ascontiguousarray(elc.T).astype(NPBF),
            shr=np.ascontiguousarray(
                shc.T.reshape(4, NT, ET).transpose(1, 0, 2).reshape(NT, 4 * ET)
            ).astype(NPBF),
            indb=np.ascontiguousarray(onehot.T).astype(NPBF),
            pk=pkc,
            krs=krs, krv1=krv1, krv2=krv2,
        )
        m.update(W)
        in_maps.append(m)
    return in_maps, core_perms


def get_packs_meta(inputs):
    pb, pf, pk8 = _pack_layouts(inputs)
    return pb, pf, pk8


def _ensure_nc(inputs):
    if "nc" not in _CACHE:
        pb, pf, pk8 = _pack_layouts(inputs)
        _CACHE["nc"] = build_nc(pb.col, pf.col, pk8.col, pb.sl, pf.sl, pk8.sl)
    return _CACHE["nc"]


def run(inputs, trace=False, tmpdir=None):
    nc = _ensure_nc(inputs)
    in_maps, core_perms = prep_inputs(inputs)
    try:
        res = run_bass_kernel_spmd(nc, in_maps, core_ids=list(range(NCORES)),
                                   trace=trace, tmpdir=tmpdir)
    except ModuleNotFoundError:
        res = run_bass_kernel_spmd(nc, in_maps, core_ids=list(range(NCORES)), trace=False)
    out = np.empty((E, DIM), np.float32)
    for c in range(NCORES):
        pidx = core_perms[c]
        blk = np.asarray(res.results[c]["out_fm"], dtype=np.float32)  # [128,3,EPC]
        ne = len(pidx)
        rows = np.empty((ne, DIM), np.float32)
        rows[:, 0:128] = blk[:, 0, :ne].T
        rows[:, 128:256] = blk[:, 1, :ne].T
        rows[:, 256:320] = blk[0:64, 2, :ne].T
        # m-major -> interleaved (v, m)
        s = rows[:, :NS]
        v = rows[:, NS:].reshape(ne, 3, NV).transpose(0, 2, 1)
        out[pidx] = np.concatenate([s, v.reshape(ne, -1)], axis=1)
    return out, res


def kernel(**inputs) -> np.ndarray:
    out, _ = run(inputs, trace=False)
    return out
